# revision 9
# baseline (speedup 1.0000x reference)
"""Cumulative link (ordinal) loss on 8 Trainium2 NeuronCores.

loss = mean_i [ -ln( sigmoid(hi_i - x_i) - sigmoid(lo_i - x_i) + eps ) ]
with per-label thresholds hi = [0,1,2,3,+inf][l], lo = [-inf,0,1,2,3][l].

Branch-free device formulation (l = label as float, G = l - x):
    S1 = sigmoid(G)          # = sigmoid(hi - x) when l <= 3
    S2 = sigmoid(G - 1)      # = sigmoid(lo - x) when l >= 1
    A  = max(l - 3, S1)      # l==4  ->  1,  else S1
    B  = min(l, S2)          # l==0  ->  0,  else S2
    p  = A - B
    partial = sum_free ln(p + eps)       (ACT Ln with accum_out)
Host: loss = -sum(partials) / B.

Perf notes:
  * Labels are DMAd straight from their int64 DRAM form with an
    in-flight SWDGE cast to dense fp16 (contiguous descriptors, full
    line rate); logits are cast f32->fp16 in-flight the same way.
  * Every DVE elementwise op is fp16-dense so the 2x_1P perf mode
    engages (the l-3 mask uses a single-src tensor_scalar at 4x).
    fp16 keeps numerics safe: ~1e-5 rel err end to end (bf16
    S-values would be ~1e-3, f32 everywhere runs the DVE at 1x).
  * In-place chains: F3 lands in x16's slot, A in G's, B in S1's,
    P in S2's; the final chunked Ln runs in place over P with
    per-chunk accumulators, dep-forced after all sigmoids so the
    activation table switches exactly once.

Sharding: pure data parallel, 1/8 of the batch per core, laid out
[128 partitions x 8192 free].
"""

import numpy as np

B_TOTAL = 8388608
N_CORES = 8
P = 128
SHARD = B_TOTAL // N_CORES          # 1048576 per core
M = SHARD // P                      # 8192 free-dim columns per core
T = 2048                            # tile width (columns)
NT = M // T
H = M // 2                          # Ln chunk width
EPS = 1e-8

_NC = None


def _build_nc():
    import concourse.bacc as bacc
    import concourse.mybir as mybir
    from concourse import tile
    from concourse.tile_rust import add_dep_helper

    f32 = mybir.dt.float32
    f16 = mybir.dt.float16
    i32 = mybir.dt.int32
    i64 = mybir.dt.int64
    Alu = mybir.AluOpType
    Act = mybir.ActivationFunctionType

    nc = bacc.Bacc("TRN2", target_bir_lowering=False, debug=False)

    x_dram = nc.dram_tensor("logits", (P, M), f32, kind="ExternalInput")
    # int32 pairs at the PJRT boundary (int64 inputs crash the axon run
    # path); bitcast back to int64 in-kernel for the casting DMA
    l_dram = nc.dram_tensor("labels", (P, 2 * M), i32, kind="ExternalInput")
    o_dram = nc.dram_tensor("out", (P, 2), f32, kind="ExternalOutput")
    l64 = l_dram[:].bitcast(i64)            # (P, M) int64 view

    def ts(t, w=T):
        return slice(t * w, (t + 1) * w)

    with tile.TileContext(nc) as tc:
        with tc.tile_pool(name="io", bufs=3) as iop, \
             tc.tile_pool(name="persist", bufs=1) as pp:
            bias_m1 = pp.tile([P, 1], f32, tag="bias_m1")
            nc.vector.memset(bias_m1[:], -1.0)
            bias_eps = pp.tile([P, 1], f32, tag="bias_eps")
            nc.vector.memset(bias_eps[:], EPS)

            g_full = pp.tile([P, M], f16, tag="g_full")    # G, then A
            s1_full = pp.tile([P, M], f16, tag="s1_full")  # S1, then B
            s2_full = pp.tile([P, M], f16, tag="s2_full")  # S2, then P, then ln
            acc = pp.tile([P, 2], f32, tag="acc")

            sigs = []
            lns = []
            x16s, l32s, levs = [], [], []
            # issue every DMA before any GpSimd cast op so SWDGE descriptor
            # generation is not blocked behind compute on the Pool queue
            for t in range(NT):
                x16 = iop.tile([P, T], f16, tag="x16")
                l32 = iop.tile([P, T, 2], i32, tag="l32")
                nc.gpsimd.dma_start(out=x16[:], in_=x_dram[:, ts(t)])   # cast f32->fp16
                nc.sync.dma_start(out=l32[:], in_=l_dram[:, ts(t, 2 * T)])
                x16s.append(x16); l32s.append(l32)
            for t in range(NT):
                lev = pp.tile([P, T], f16, tag=f"lev{t}")
                # int32 low words (stride 2) -> dense fp16 on idle GpSimd
                nc.gpsimd.tensor_copy(out=lev[:], in_=l32s[t][:, :, 0])
                levs.append(lev)
            for t in range(NT):
                x16, lev = x16s[t], levs[t]
                g = g_full[:, ts(t)]
                s1 = s1_full[:, ts(t)]
                s2 = s2_full[:, ts(t)]
                # G = l - x                       (fp16 TT, 2x)
                nc.vector.tensor_tensor(out=g, in0=lev[:], in1=x16[:],
                                        op=Alu.subtract)
                sigs.append(nc.scalar.activation(s1, g, Act.Sigmoid))
                sigs.append(
                    nc.scalar.activation(s2, g, Act.Sigmoid, bias=bias_m1[:])
                )
                # F3 = l - 3 -> x16's slot        (fp16 TS, 4x)
                nc.vector.tensor_scalar_sub(x16[:], lev[:], 3.0)
                # A = max(F3, S1) -> G's slot     (fp16 TT, 2x)
                nc.vector.tensor_max(g, x16[:], s1)
                # B = min(l, S2) -> S1's slot     (fp16 TT, 2x)
                nc.vector.tensor_tensor(out=s1, in0=lev[:], in1=s2, op=Alu.min)
                # P = A - B -> S2's slot          (fp16 TT, 2x)
                nc.vector.tensor_tensor(out=s2, in0=g, in1=s1, op=Alu.subtract)

            # ln(P + eps) in place, one accumulator column per half
            for h in range(2):
                lns.append(
                    nc.scalar.activation(
                        s2_full[:, ts(h, H)], s2_full[:, ts(h, H)], Act.Ln,
                        bias=bias_eps[:], accum_out=acc[:, h:h + 1],
                    )
                )
            # every sigmoid precedes both Lns -> one act-table switch
            for ln in lns:
                for sg in sigs:
                    add_dep_helper(
                        ln.ins, sg.ins, sync=False, reason="one act-table switch"
                    )
            nc.sync.dma_start(out=o_dram[:], in_=acc[:])

    nc.compile()
    return nc


def get_nc():
    global _NC
    if _NC is None:
        _NC = _build_nc()
    return _NC


def make_in_maps(logits, labels):
    x = np.ascontiguousarray(np.asarray(logits, dtype=np.float32)).reshape(B_TOTAL)
    lab = np.asarray(labels)
    if lab.dtype != np.int64:
        lab = lab.astype(np.int64)
    lab = np.ascontiguousarray(lab).reshape(B_TOTAL)
    in_maps = []
    for c in range(N_CORES):
        xs = x[c * SHARD:(c + 1) * SHARD].reshape(P, M)
        ls = lab[c * SHARD:(c + 1) * SHARD].view(np.int32).reshape(P, 2 * M)
        in_maps.append({"logits": xs, "labels": ls})
    return in_maps


def run(logits, labels, trace=False):
    """Returns (loss_scalar_f32, BassKernelResults)."""
    from concourse.bass_utils import run_bass_kernel_spmd

    nc = get_nc()
    in_maps = make_in_maps(logits, labels)
    res = run_bass_kernel_spmd(
        nc, in_maps, core_ids=list(range(N_CORES)), trace=trace
    )
    total = 0.0
    for r in res.results:
        total += r["out"].astype(np.float64).sum()
    loss = np.float32(-total / B_TOTAL)
    return np.asarray(loss), res


def kernel(logits, labels):
    out, _ = run(logits, labels, trace=False)
    return out


# revision 10
# speedup vs baseline: 1.1563x; 1.1563x over previous
"""Cumulative link (ordinal) loss on 8 Trainium2 NeuronCores.

loss = mean_i [ -ln( sigmoid(hi_i - x_i) - sigmoid(lo_i - x_i) + eps ) ]
with per-label thresholds hi = [0,1,2,3,+inf][l], lo = [-inf,0,1,2,3][l].

Branch-free device formulation (l = label as float, G = l - x):
    S1 = sigmoid(G)          # = sigmoid(hi - x) when l <= 3
    S2 = sigmoid(G - 1)      # = sigmoid(lo - x) when l >= 1
    A  = max(l - 3, S1)      # l==4  ->  1,  else S1
    B  = min(l, S2)          # l==0  ->  0,  else S2
    p  = A - B
    partial = sum_free ln(p + eps)       (ACT Ln with accum_out)
Host: loss = -sum(partials) / B.

Perf notes:
  * Labels are DMAd straight from their int64 DRAM form with an
    in-flight SWDGE cast to dense fp16 (contiguous descriptors, full
    line rate); logits are cast f32->fp16 in-flight the same way.
  * Every DVE elementwise op is fp16-dense so the 2x_1P perf mode
    engages (the l-3 mask uses a single-src tensor_scalar at 4x).
    fp16 keeps numerics safe: ~1e-5 rel err end to end (bf16
    S-values would be ~1e-3, f32 everywhere runs the DVE at 1x).
  * In-place chains: F3 lands in x16's slot, A in G's, B in S1's,
    P in S2's; the final chunked Ln runs in place over P with
    per-chunk accumulators, dep-forced after all sigmoids so the
    activation table switches exactly once.

Sharding: pure data parallel, 1/8 of the batch per core, laid out
[128 partitions x 8192 free].
"""

import numpy as np

B_TOTAL = 8388608
N_CORES = 8
P = 128
SHARD = B_TOTAL // N_CORES          # 1048576 per core
M = SHARD // P                      # 8192 free-dim columns per core
T = 2048                            # tile width (columns)
NT = M // T
H = M // 2                          # Ln chunk width
EPS = 1e-8

_NC = None


def _build_nc():
    import concourse.bacc as bacc
    import concourse.mybir as mybir
    from concourse import tile
    from concourse.tile_rust import add_dep_helper

    f32 = mybir.dt.float32
    f16 = mybir.dt.float16
    i32 = mybir.dt.int32
    i64 = mybir.dt.int64
    Alu = mybir.AluOpType
    Act = mybir.ActivationFunctionType

    nc = bacc.Bacc("TRN2", target_bir_lowering=False, debug=False)

    x_dram = nc.dram_tensor("logits", (P, M), f32, kind="ExternalInput")
    # int32 pairs at the PJRT boundary (int64 inputs crash the axon run
    # path); bitcast back to int64 in-kernel for the casting DMA
    l_dram = nc.dram_tensor("labels", (P, 2 * M), i32, kind="ExternalInput")
    o_dram = nc.dram_tensor("out", (P, 2), f32, kind="ExternalOutput")
    l64 = l_dram[:].bitcast(i64)            # (P, M) int64 view

    def ts(t, w=T):
        return slice(t * w, (t + 1) * w)

    with tile.TileContext(nc) as tc:
        with tc.tile_pool(name="io", bufs=3) as iop, \
             tc.tile_pool(name="persist", bufs=1) as pp:
            bias_m1 = pp.tile([P, 1], f32, tag="bias_m1")
            nc.vector.memset(bias_m1[:], -1.0)
            bias_eps = pp.tile([P, 1], f32, tag="bias_eps")
            nc.vector.memset(bias_eps[:], EPS)

            g_full = pp.tile([P, M], f16, tag="g_full")    # G, then A
            s1_full = pp.tile([P, M], f16, tag="s1_full")  # S1, then B
            s2_full = pp.tile([P, M], f16, tag="s2_full")  # S2, then P, then ln
            acc = pp.tile([P, 2], f32, tag="acc")

            sigs = []
            lns = []
            x16s, l32s, levs = [], [], []
            # issue every DMA before any GpSimd cast op so SWDGE descriptor
            # generation is not blocked behind compute on the Pool queue
            for t in range(NT):
                x16 = iop.tile([P, T], f16, tag="x16")
                l32 = iop.tile([P, T, 2], i32, tag="l32")
                nc.gpsimd.dma_start(out=x16[:], in_=x_dram[:, ts(t)])   # cast f32->fp16
                nc.sync.dma_start(out=l32[:], in_=l_dram[:, ts(t, 2 * T)])
                x16s.append(x16); l32s.append(l32)
            for t in range(NT):
                lev = pp.tile([P, T], f16, tag=f"lev{t}")
                # int32 low words (stride 2) -> dense fp16 (DVE; GpSimd's
                # CAST stalls concurrent DVE ops via the shared SBUF port)
                nc.vector.tensor_copy(out=lev[:], in_=l32s[t][:, :, 0])
                levs.append(lev)
            for t in range(NT):
                x16, lev = x16s[t], levs[t]
                g = g_full[:, ts(t)]
                s1 = s1_full[:, ts(t)]
                s2 = s2_full[:, ts(t)]
                # G = l - x                       (fp16 TT, 2x)
                nc.vector.tensor_tensor(out=g, in0=lev[:], in1=x16[:],
                                        op=Alu.subtract)
                sigs.append(nc.scalar.activation(s1, g, Act.Sigmoid))
                sigs.append(
                    nc.scalar.activation(s2, g, Act.Sigmoid, bias=bias_m1[:])
                )
                # F3 = l - 3 -> x16's slot        (fp16 TS, 4x)
                nc.vector.tensor_scalar_sub(x16[:], lev[:], 3.0)
                # A = max(F3, S1) -> G's slot     (fp16 TT, 2x)
                nc.vector.tensor_max(g, x16[:], s1)
                # B = min(l, S2) -> S1's slot     (fp16 TT, 2x)
                nc.vector.tensor_tensor(out=s1, in0=lev[:], in1=s2, op=Alu.min)
                # P = A - B -> S2's slot          (fp16 TT, 2x)
                nc.vector.tensor_tensor(out=s2, in0=g, in1=s1, op=Alu.subtract)

            # ln(P + eps) in place, one accumulator column per half
            for h in range(2):
                lns.append(
                    nc.scalar.activation(
                        s2_full[:, ts(h, H)], s2_full[:, ts(h, H)], Act.Ln,
                        bias=bias_eps[:], accum_out=acc[:, h:h + 1],
                    )
                )
            # every sigmoid precedes both Lns -> one act-table switch
            for ln in lns:
                for sg in sigs:
                    add_dep_helper(
                        ln.ins, sg.ins, sync=False, reason="one act-table switch"
                    )
            nc.sync.dma_start(out=o_dram[:], in_=acc[:])

    nc.compile()
    return nc


def get_nc():
    global _NC
    if _NC is None:
        _NC = _build_nc()
    return _NC


def make_in_maps(logits, labels):
    x = np.ascontiguousarray(np.asarray(logits, dtype=np.float32)).reshape(B_TOTAL)
    lab = np.asarray(labels)
    if lab.dtype != np.int64:
        lab = lab.astype(np.int64)
    lab = np.ascontiguousarray(lab).reshape(B_TOTAL)
    in_maps = []
    for c in range(N_CORES):
        xs = x[c * SHARD:(c + 1) * SHARD].reshape(P, M)
        ls = lab[c * SHARD:(c + 1) * SHARD].view(np.int32).reshape(P, 2 * M)
        in_maps.append({"logits": xs, "labels": ls})
    return in_maps


def run(logits, labels, trace=False):
    """Returns (loss_scalar_f32, BassKernelResults)."""
    from concourse.bass_utils import run_bass_kernel_spmd

    nc = get_nc()
    in_maps = make_in_maps(logits, labels)
    res = run_bass_kernel_spmd(
        nc, in_maps, core_ids=list(range(N_CORES)), trace=trace
    )
    total = 0.0
    for r in res.results:
        total += r["out"].astype(np.float64).sum()
    loss = np.float32(-total / B_TOTAL)
    return np.asarray(loss), res


def kernel(logits, labels):
    out, _ = run(logits, labels, trace=False)
    return out


# revision 12
# speedup vs baseline: 1.2075x; 1.0443x over previous
"""Cumulative link (ordinal) loss on 8 Trainium2 NeuronCores.

loss = mean_i [ -ln( sigmoid(hi_i - x_i) - sigmoid(lo_i - x_i) + eps ) ]
with per-label thresholds hi = [0,1,2,3,+inf][l], lo = [-inf,0,1,2,3][l].

Branch-free device formulation (l = label as float, G = l - x):
    S1 = sigmoid(G)          # = sigmoid(hi - x) when l <= 3
    S2 = sigmoid(G - 1)      # = sigmoid(lo - x) when l >= 1
    A  = max(l - 3, S1)      # l==4  ->  1,  else S1
    B  = min(l, S2)          # l==0  ->  0,  else S2
    p  = A - B
    partial = sum_free ln(p + eps)       (ACT Ln with accum_out)
Host: loss = -sum(partials) / B.

Perf notes:
  * Labels are DMAd straight from their int64 DRAM form with an
    in-flight SWDGE cast to dense fp16 (contiguous descriptors, full
    line rate); logits are cast f32->fp16 in-flight the same way.
  * Every DVE elementwise op is fp16-dense so the 2x_1P perf mode
    engages (the l-3 mask uses a single-src tensor_scalar at 4x).
    fp16 keeps numerics safe: ~1e-5 rel err end to end (bf16
    S-values would be ~1e-3, f32 everywhere runs the DVE at 1x).
  * In-place chains: F3 lands in x16's slot, A in G's, B in S1's,
    P in S2's; the final chunked Ln runs in place over P with
    per-chunk accumulators, dep-forced after all sigmoids so the
    activation table switches exactly once.

Sharding: pure data parallel, 1/8 of the batch per core, laid out
[128 partitions x 8192 free].
"""

import numpy as np

B_TOTAL = 8388608
N_CORES = 8
P = 128
SHARD = B_TOTAL // N_CORES          # 1048576 per core
M = SHARD // P                      # 8192 free-dim columns per core
T = 2048                            # tile width (columns)
NT = M // T
H = M // 2                          # Ln chunk width
EPS = 1e-8

_NC = None


def _build_nc():
    import concourse.bacc as bacc
    import concourse.mybir as mybir
    from concourse import tile
    from concourse.tile_rust import add_dep_helper

    f32 = mybir.dt.float32
    f16 = mybir.dt.float16
    i32 = mybir.dt.int32
    i64 = mybir.dt.int64
    Alu = mybir.AluOpType
    Act = mybir.ActivationFunctionType

    nc = bacc.Bacc("TRN2", target_bir_lowering=False, debug=False,
                   enable_asserts=False)

    x_dram = nc.dram_tensor("logits", (P, M), f32, kind="ExternalInput")
    # int32 pairs at the PJRT boundary (int64 inputs crash the axon run
    # path); bitcast back to int64 in-kernel for the casting DMA
    l_dram = nc.dram_tensor("labels", (P, 2 * M), i32, kind="ExternalInput")
    o_dram = nc.dram_tensor("out", (P, NT), f32, kind="ExternalOutput")
    l64 = l_dram[:].bitcast(i64)            # (P, M) int64 view

    def ts(t, w=T):
        return slice(t * w, (t + 1) * w)

    with tile.TileContext(nc) as tc:
        with tc.tile_pool(name="io", bufs=3) as iop, \
             tc.tile_pool(name="persist", bufs=1) as pp:
            bias_m1 = pp.tile([P, 1], f32, tag="bias_m1")
            nc.vector.memset(bias_m1[:], -1.0)
            bias_eps = pp.tile([P, 1], f32, tag="bias_eps")
            nc.vector.memset(bias_eps[:], EPS)

            g_full = pp.tile([P, M], f16, tag="g_full")    # G, then A
            s1_full = pp.tile([P, M], f16, tag="s1_full")  # S1, then B
            s2_full = pp.tile([P, M], f16, tag="s2_full")  # S2, then P, then ln
            acc = pp.tile([P, NT], f32, tag="acc")

            sigs = []
            lns = []
            x16s, l32s, levs = [], [], []
            # issue every DMA before any GpSimd cast op so SWDGE descriptor
            # generation is not blocked behind compute on the Pool queue
            for t in range(NT):
                x16 = iop.tile([P, T], f16, tag="x16")
                l32 = iop.tile([P, T, 2], i32, tag="l32")
                nc.gpsimd.dma_start(out=x16[:], in_=x_dram[:, ts(t)])   # cast f32->fp16
                nc.sync.dma_start(out=l32[:], in_=l_dram[:, ts(t, 2 * T)])
                x16s.append(x16); l32s.append(l32)
            for t in range(NT):
                lev = pp.tile([P, T], f16, tag=f"lev{t}")
                # int32 low words (stride 2) -> dense fp16 (DVE; GpSimd's
                # CAST stalls concurrent DVE ops via the shared SBUF port)
                nc.vector.tensor_copy(out=lev[:], in_=l32s[t][:, :, 0])
                levs.append(lev)
            for t in range(NT):
                x16, lev = x16s[t], levs[t]
                g = g_full[:, ts(t)]
                s1 = s1_full[:, ts(t)]
                s2 = s2_full[:, ts(t)]
                # G = l - x                       (fp16 TT, 2x)
                nc.vector.tensor_tensor(out=g, in0=lev[:], in1=x16[:],
                                        op=Alu.subtract)
                sigs.append(nc.scalar.activation(s1, g, Act.Sigmoid))
                sigs.append(
                    nc.scalar.activation(s2, g, Act.Sigmoid, bias=bias_m1[:])
                )
                # F3 = l - 3 -> x16's slot        (fp16 TS, 4x)
                nc.vector.tensor_scalar_sub(x16[:], lev[:], 3.0)
                # A = max(F3, S1) -> G's slot     (fp16 TT, 2x)
                nc.vector.tensor_max(g, x16[:], s1)
                # B = min(l, S2) -> S1's slot     (fp16 TT, 2x)
                nc.vector.tensor_tensor(out=s1, in0=lev[:], in1=s2, op=Alu.min)
                # P = A - B -> S2's slot          (fp16 TT, 2x)
                nc.vector.tensor_tensor(out=s2, in0=g, in1=s1, op=Alu.subtract)

            # ln(P + eps) per tile, in place, one accumulator column each.
            for t in range(NT):
                lns.append(
                    nc.scalar.activation(
                        s2_full[:, ts(t)], s2_full[:, ts(t)], Act.Ln,
                        bias=bias_eps[:], accum_out=acc[:, t:t + 1],
                    )
                )
            # Pin the ACT program order so Ln chunks run inside the ACT
            # engine's DMA-gated idle windows instead of queuing after the
            # last sigmoid:  s0 s0' s1 s1' ln0 s2 s2' ln1 s3 s3' ln2 ln3.
            act_order = (sigs[0:4] + [lns[0]] + sigs[4:6] + [lns[1]]
                         + sigs[6:8] + [lns[2], lns[3]])
            for prev, nxt in zip(act_order, act_order[1:]):
                add_dep_helper(nxt.ins, prev.ins, sync=False,
                               reason="pin ACT order")
            nc.sync.dma_start(out=o_dram[:], in_=acc[:])

    nc.compile()
    return nc


def get_nc():
    global _NC
    if _NC is None:
        _NC = _build_nc()
    return _NC


def make_in_maps(logits, labels):
    x = np.ascontiguousarray(np.asarray(logits, dtype=np.float32)).reshape(B_TOTAL)
    lab = np.asarray(labels)
    if lab.dtype != np.int64:
        lab = lab.astype(np.int64)
    lab = np.ascontiguousarray(lab).reshape(B_TOTAL)
    in_maps = []
    for c in range(N_CORES):
        xs = x[c * SHARD:(c + 1) * SHARD].reshape(P, M)
        ls = lab[c * SHARD:(c + 1) * SHARD].view(np.int32).reshape(P, 2 * M)
        in_maps.append({"logits": xs, "labels": ls})
    return in_maps


def run(logits, labels, trace=False):
    """Returns (loss_scalar_f32, BassKernelResults)."""
    from concourse.bass_utils import run_bass_kernel_spmd

    nc = get_nc()
    in_maps = make_in_maps(logits, labels)
    res = run_bass_kernel_spmd(
        nc, in_maps, core_ids=list(range(N_CORES)), trace=trace
    )
    total = 0.0
    for r in res.results:
        total += r["out"].astype(np.float64).sum()
    loss = np.float32(-total / B_TOTAL)
    return np.asarray(loss), res


def kernel(logits, labels):
    out, _ = run(logits, labels, trace=False)
    return out


# revision 13
# speedup vs baseline: 1.3093x; 1.0843x over previous
"""Cumulative link (ordinal) loss on 8 Trainium2 NeuronCores.

loss = mean_i [ -ln( sigmoid(hi_i - x_i) - sigmoid(lo_i - x_i) + eps ) ]
with per-label thresholds hi = [0,1,2,3,+inf][l], lo = [-inf,0,1,2,3][l].

Branch-free device formulation (l = label as float, G = l - x):
    S1 = sigmoid(G)          # = sigmoid(hi - x) when l <= 3
    S2 = sigmoid(G - 1)      # = sigmoid(lo - x) when l >= 1
    A  = max(l - 3, S1)      # l==4  ->  1,  else S1
    B  = min(l, S2)          # l==0  ->  0,  else S2
    p  = A - B
    partial = sum_free ln(p + eps)       (ACT Ln with accum_out)
Host: loss = -sum(partials) / B.

Perf notes:
  * Labels are DMAd straight from their int64 DRAM form with an
    in-flight SWDGE cast to dense fp16 (contiguous descriptors, full
    line rate); logits are cast f32->fp16 in-flight the same way.
  * Every DVE elementwise op is fp16-dense so the 2x_1P perf mode
    engages (the l-3 mask uses a single-src tensor_scalar at 4x).
    fp16 keeps numerics safe: ~1e-5 rel err end to end (bf16
    S-values would be ~1e-3, f32 everywhere runs the DVE at 1x).
  * In-place chains: F3 lands in x16's slot, A in G's, B in S1's,
    P in S2's; the final chunked Ln runs in place over P with
    per-chunk accumulators, dep-forced after all sigmoids so the
    activation table switches exactly once.

Sharding: pure data parallel, 1/8 of the batch per core, laid out
[128 partitions x 8192 free].
"""

import numpy as np

B_TOTAL = 8388608
N_CORES = 8
P = 128
SHARD = B_TOTAL // N_CORES          # 1048576 per core
M = SHARD // P                      # 8192 free-dim columns per core
T = 2048                            # tile width (columns)
NT = M // T
H = M // 2                          # Ln chunk width
EPS = 1e-8

_NC = None


def _build_nc():
    import concourse.bacc as bacc
    import concourse.mybir as mybir
    from concourse import tile
    from concourse.tile_rust import add_dep_helper

    f32 = mybir.dt.float32
    f16 = mybir.dt.float16
    i32 = mybir.dt.int32
    i64 = mybir.dt.int64
    Alu = mybir.AluOpType
    Act = mybir.ActivationFunctionType

    nc = bacc.Bacc("TRN2", target_bir_lowering=False, debug=False,
                   enable_asserts=False)

    x_dram = nc.dram_tensor("logits", (P, M), f32, kind="ExternalInput")
    # int32 pairs at the PJRT boundary (int64 inputs crash the axon run
    # path); bitcast back to int64 in-kernel for the casting DMA
    l_dram = nc.dram_tensor("labels", (P, 2 * M), i32, kind="ExternalInput")
    o_dram = nc.dram_tensor("out", (P, NT), f32, kind="ExternalOutput")
    l64 = l_dram[:].bitcast(i64)            # (P, M) int64 view

    def ts(t, w=T):
        return slice(t * w, (t + 1) * w)

    with tile.TileContext(nc) as tc:
        with tc.tile_pool(name="io", bufs=3) as iop, \
             tc.tile_pool(name="persist", bufs=1) as pp:
            bias_m1 = pp.tile([P, 1], f32, tag="bias_m1")
            nc.vector.memset(bias_m1[:], -1.0)
            bias_eps = pp.tile([P, 1], f32, tag="bias_eps")
            nc.vector.memset(bias_eps[:], EPS)

            g_full = pp.tile([P, M], f16, tag="g_full")    # G, then A
            s1_full = pp.tile([P, M], f16, tag="s1_full")  # S1, then B
            s2_full = pp.tile([P, M], f16, tag="s2_full")  # S2, then P, then ln
            acc = pp.tile([P, NT], f32, tag="acc")

            sigs = []
            lns = []
            x16s, l32s, levs = [], [], []
            # issue every DMA before any GpSimd cast op so SWDGE descriptor
            # generation is not blocked behind compute on the Pool queue
            for t in range(NT):
                x16 = iop.tile([P, T], f16, tag="x16")
                l32 = iop.tile([P, T, 2], i32, tag="l32")
                nc.gpsimd.dma_start(out=x16[:], in_=x_dram[:, ts(t)])   # cast f32->fp16
                nc.sync.dma_start(out=l32[:], in_=l_dram[:, ts(t, 2 * T)])
                x16s.append(x16); l32s.append(l32)
            for t in range(NT):
                x16 = x16s[t]
                lev = pp.tile([P, T], f16, tag=f"lev{t}")
                # int32 low words (stride 2) -> dense fp16 (DVE; GpSimd's
                # CAST stalls concurrent DVE ops via the shared SBUF port).
                # Emitted per tile so scheduler priorities follow tile order.
                nc.vector.tensor_copy(out=lev[:], in_=l32s[t][:, :, 0])
                g = g_full[:, ts(t)]
                s1 = s1_full[:, ts(t)]
                s2 = s2_full[:, ts(t)]
                # G = l - x                       (fp16 TT, 2x)
                nc.vector.tensor_tensor(out=g, in0=lev[:], in1=x16[:],
                                        op=Alu.subtract)
                sigs.append(nc.scalar.activation(s1, g, Act.Sigmoid))
                sigs.append(
                    nc.scalar.activation(s2, g, Act.Sigmoid, bias=bias_m1[:])
                )
                # F3 = l - 3 -> x16's slot        (fp16 TS, 4x)
                nc.vector.tensor_scalar_sub(x16[:], lev[:], 3.0)
                # A = max(F3, S1) -> G's slot     (fp16 TT, 2x)
                nc.vector.tensor_max(g, x16[:], s1)
                # B = min(l, S2) -> S1's slot     (fp16 TT, 2x)
                nc.vector.tensor_tensor(out=s1, in0=lev[:], in1=s2, op=Alu.min)
                # P = A - B -> S2's slot          (fp16 TT, 2x)
                nc.vector.tensor_tensor(out=s2, in0=g, in1=s1, op=Alu.subtract)

            # ln(P + eps) per tile, in place, one accumulator column each.
            for t in range(NT):
                lns.append(
                    nc.scalar.activation(
                        s2_full[:, ts(t)], s2_full[:, ts(t)], Act.Ln,
                        bias=bias_eps[:], accum_out=acc[:, t:t + 1],
                    )
                )
            # Pin the ACT program order so Ln chunks run inside the ACT
            # engine's DMA-gated idle windows instead of queuing after the
            # last sigmoid:  s0 s0' s1 s1' ln0 s2 s2' ln1 s3 s3' ln2 ln3.
            act_order = (sigs[0:4] + [lns[0]] + sigs[4:6] + [lns[1]]
                         + sigs[6:8] + [lns[2], lns[3]])
            for prev, nxt in zip(act_order, act_order[1:]):
                add_dep_helper(nxt.ins, prev.ins, sync=False,
                               reason="pin ACT order")
            nc.sync.dma_start(out=o_dram[:], in_=acc[:])

    nc.compile()
    return nc


def get_nc():
    global _NC
    if _NC is None:
        _NC = _build_nc()
    return _NC


def make_in_maps(logits, labels):
    x = np.ascontiguousarray(np.asarray(logits, dtype=np.float32)).reshape(B_TOTAL)
    lab = np.asarray(labels)
    if lab.dtype != np.int64:
        lab = lab.astype(np.int64)
    lab = np.ascontiguousarray(lab).reshape(B_TOTAL)
    in_maps = []
    for c in range(N_CORES):
        xs = x[c * SHARD:(c + 1) * SHARD].reshape(P, M)
        ls = lab[c * SHARD:(c + 1) * SHARD].view(np.int32).reshape(P, 2 * M)
        in_maps.append({"logits": xs, "labels": ls})
    return in_maps


def run(logits, labels, trace=False):
    """Returns (loss_scalar_f32, BassKernelResults)."""
    from concourse.bass_utils import run_bass_kernel_spmd

    nc = get_nc()
    in_maps = make_in_maps(logits, labels)
    res = run_bass_kernel_spmd(
        nc, in_maps, core_ids=list(range(N_CORES)), trace=trace
    )
    total = 0.0
    for r in res.results:
        total += r["out"].astype(np.float64).sum()
    loss = np.float32(-total / B_TOTAL)
    return np.asarray(loss), res


def kernel(logits, labels):
    out, _ = run(logits, labels, trace=False)
    return out
